# revision 14
# baseline (speedup 1.0000x reference)
"""Multi-head attention forward on 8 TRN2 NeuronCores (data-parallel over batch).

Reference computation (B=64, T=197, D=768, H=12, DK=64, fp32):
    q = split_heads(x @ Wq + bq); k = ...; v = ...
    scores = floor((q @ k^T) / 8); attn = softmax(scores); out = attn @ v
    return merge_heads(out) @ Wo + bo

Numerics: floor() before softmax makes the Q/K path extremely sensitive.
It runs as error-corrected fp16 matmuls: every fp32 operand a is split as
a = a_hi + a_lo (fp16 halves) and a@b = a_hi@b_hi + a_hi@b_lo + a_lo@b_hi
(the lo@lo term is negligible).  Products are exact in fp32 PSUM, so this
is slightly MORE accurate than native fp32 matmul (validated on HW:
2.6e-5 vs 6e-5 abs err vs fp64) at 3 cycles/row instead of 4.  The V path
(v proj, attn@v, out proj) runs in plain fp16 (1 cyc/row).

Per-core dataflow (8 batch elements each, all-transposed activations):
  P0:  x row-chunks DMA'd, PE-transposed; DVE splits them into
       x_hi/x_lo fp16 tiles [128,1576].
  P1a: q = (Wq/8)^T @ x, k = Wk^T @ x via 3-term fp16 (weights pre-split
       on host); PSUM fp32 results split into q_hi/q_lo/k_hi/k_lo fp16.
  P1b: v16e[b,kc][keys,12*65] = x_hi^T @ Wv16 (fp16), heads strided by
       65 with a ones column per head (memset first) so attn@v also
       produces the softmax denominator.
  P2 (per b, head-pair): scoresT[keys,197] = kT.T @ qT (3-term fp16,
      2 heads row-packed via tile_position, separate PSUM tiles --
      sharing one tile faults the HW); floor via round_half_even(x-0.5)
      (DVE magic-number add, magic 1.5*2^23) with the -magic correction
      folded into the ScalarE Exp bias; attn@v transposed ->
      outT[0:64]=out, outT[64]=denominator. Denominators for all 12
      heads are gathered into one tile (partition base 32*(h%4), col
      block h//4 -- DVE writes must start at partition 0/32/64/96),
      one batched reciprocal, then per-pair PE-broadcast + DVE multiply.
  P3 (per b): final = outT16^T @ Wo16, DVE copy to SBUF, DMA out.

Bias matmuls (K=1 ones-row) are only emitted when any bias is nonzero;
the build is specialized on that flag.  All PSUM tiles come from one
shared-tag pool (8 banks round-robin) so phases overlap freely.
"""

import numpy as np
import ml_dtypes

B, T, D, H, DK = 64, 197, 768, 12, 64
NCORES = 8
BL = B // NCORES          # 8 batch elements per core
R = BL * T                # 1576 rows per core
ND = D // 128             # 6 chunks of 128 along D
NC4 = 4                   # proj col chunks
CW = R // NC4             # 394
HV = DK + 1               # 65: per-head v stride (ones column at 64)
ROWCHUNKS = [(i * 128, min(128, R - i * 128)) for i in range((R + 127) // 128)]
KEYCHUNKS = [(0, 128), (128, 69)]
MAGIC = float(3 * 2 ** 22)  # 1.5*2^23: x-0.5+MAGIC stays in [2^23,2^24), ulp=1

_CACHE = {}


def _build(has_bias):
    import concourse.bacc as bacc
    import concourse.mybir as mybir
    import concourse.tile as tile
    from concourse.masks import make_identity

    f32 = mybir.dt.float32
    f16 = mybir.dt.float16
    AF = mybir.ActivationFunctionType
    OP = mybir.AluOpType

    nc = bacc.Bacc("TRN2", target_bir_lowering=False, debug=False,
                   num_devices=NCORES)

    x_d = nc.dram_tensor("x", [R, D], f32, kind="ExternalInput").ap()
    w_d = {}
    for nm in ("wq_hi", "wq_lo", "wk_hi", "wk_lo", "wv", "wo"):
        w_d[nm] = nc.dram_tensor(nm, [D, D], f16, kind="ExternalInput").ap()
    if has_bias:
        bq_d = nc.dram_tensor("bq", [1, D], f16, kind="ExternalInput").ap()
        bk_d = nc.dram_tensor("bk", [1, D], f16, kind="ExternalInput").ap()
        bv_d = nc.dram_tensor("bv", [1, D], f16, kind="ExternalInput").ap()
        bo_d = nc.dram_tensor("bo", [1, D], f16, kind="ExternalInput").ap()
    out_d = nc.dram_tensor("out", [R, D], f32, kind="ExternalOutput").ap()

    with tile.TileContext(nc) as tc:
        with tc.tile_pool(name="static", bufs=1) as Ps, \
             tc.tile_pool(name="psum", bufs=8, space="PSUM") as Pp:

            def ptile(nm):
                return Pp.tile([128, CW], f32, name=nm, tag="ps", bufs=8,
                               uniquify=True)

            qT = [Ps.tile([128, R], f32, name=f"qT{i}") for i in range(ND)]
            kT = [Ps.tile([128, R], f32, name=f"kT{i}") for i in range(ND)]
            # v16e[2b+kc][keys<=128, 12*65]; col h*65+64 holds ones
            v16e = [Ps.tile([128, H * HV], f16, name=f"v16e_{i}")
                    for i in range(2 * BL)]
            ones_row = Ps.tile([128, CW], f16, name="ones_row")
            id32 = Ps.tile([128, 128], f32, name="id32")
            negmagic = Ps.tile([128, 1], f32, name="negmagic")

            nc.vector.memset(ones_row, 1.0)
            nc.vector.memset(negmagic, -MAGIC)
            make_identity(nc, id32)
            if has_bias:
                bq_sb = Ps.tile([1, D], f16, name="bq_sb")
                bk_sb = Ps.tile([1, D], f16, name="bk_sb")
                bv_sb = Ps.tile([1, D], f16, name="bv_sb")
                bo_sb = Ps.tile([1, D], f16, name="bo_sb")
                nc.sync.dma_start(bq_sb, bq_d)
                nc.sync.dma_start(bk_sb, bk_d)
                nc.sync.dma_start(bv_sb, bv_d)
                nc.sync.dma_start(bo_sb, bo_d)

            # ---------------- P0 + P1a: x split, q/k projections -----------
            with tc.tile_pool(name="ph1", bufs=1) as P1:
                xhi = [P1.tile([128, R], f16, name=f"xhi{i}") for i in range(ND)]
                xlo = [P1.tile([128, R], f16, name=f"xlo{i}") for i in range(ND)]
                with tc.tile_pool(name="wqk", bufs=1) as Pw:
                    wsb = {nm: [Pw.tile([128, D], f16, name=f"{nm}{k}")
                                for k in range(ND)]
                           for nm in ("wq_hi", "wq_lo", "wk_hi", "wk_lo")}
                    for nm, tiles in wsb.items():
                        for k in range(ND):
                            nc.sync.dma_start(
                                tiles[k], w_d[nm][k * 128:(k + 1) * 128, :])

                    # P0: transpose x chunks, split into hi/lo fp16
                    with tc.tile_pool(name="xst", bufs=3) as Pst:
                        for (roff, rn) in ROWCHUNKS:
                            xs = Pst.tile([128, D], f32, name="xs", tag="xs")
                            nc.sync.dma_start(xs[:rn, :], x_d[roff:roff + rn, :])
                            for d in range(ND):
                                tp = ptile("tp")
                                nc.tensor.transpose(
                                    tp[:128, :rn],
                                    xs[:rn, d * 128:(d + 1) * 128],
                                    id32[:rn, :rn])
                                hi = xhi[d][:, roff:roff + rn]
                                nc.vector.tensor_copy(hi, tp[:128, :rn])
                                nc.vector.tensor_tensor(
                                    xlo[d][:, roff:roff + rn],
                                    tp[:128, :rn], hi, OP.subtract)

                    # P1a: q/k projections, 3-term fp16; split outputs
                    for (whi, wlo, b_nm, dst) in (
                            ("wq_hi", "wq_lo", "bq", qT),
                            ("wk_hi", "wk_lo", "bk", kT)):
                        for n in range(ND):
                            ns = slice(n * 128, (n + 1) * 128)
                            for c in range(NC4):
                                cs = slice(c * CW, (c + 1) * CW)
                                pp = ptile("pp")
                                for k in range(ND):
                                    nc.tensor.matmul(
                                        pp, wsb[whi][k][:, ns], xhi[k][:, cs],
                                        start=(k == 0), stop=False)
                                    nc.tensor.matmul(
                                        pp, wsb[whi][k][:, ns], xlo[k][:, cs],
                                        start=False, stop=False)
                                    nc.tensor.matmul(
                                        pp, wsb[wlo][k][:, ns], xhi[k][:, cs],
                                        start=False,
                                        stop=(k == ND - 1 and not has_bias))
                                if has_bias:
                                    bsb = {"bq": bq_sb, "bk": bk_sb}[b_nm]
                                    nc.tensor.matmul(
                                        pp, bsb[:1, ns], ones_row[:1, :CW],
                                        start=False, stop=True)
                                nc.scalar.activation(dst[n][:, cs], pp,
                                                     AF.Copy)

                # P1b: v projection (fp16, from xhi)
                with tc.tile_pool(name="ph1b", bufs=1) as P1b:
                    wv_sb = [P1b.tile([128, D], f16, name=f"wv_sb{k}")
                             for k in range(ND)]
                    for k in range(ND):
                        nc.sync.dma_start(wv_sb[k],
                                          w_d["wv"][k * 128:(k + 1) * 128, :])
                    for i in range(2 * BL):
                        nc.vector.memset(v16e[i], 1.0)

                    for b in range(BL):
                        base = b * T
                        for kc, (koff, klen) in enumerate(KEYCHUNKS):
                            dst = v16e[2 * b + kc]
                            dst3 = dst[:klen, :].rearrange(
                                "p (h c) -> p h c", c=HV)[:, :, 0:DK]
                            for half in range(2):
                                c0 = half * 384
                                vp = ptile("vp")
                                vps = vp[:klen, :384]
                                for d in range(ND):
                                    nc.tensor.matmul(
                                        vps,
                                        xhi[d][:, base + koff:
                                               base + koff + klen],
                                        wv_sb[d][:, c0:c0 + 384],
                                        start=(d == 0),
                                        stop=(d == ND - 1 and not has_bias))
                                if has_bias:
                                    nc.tensor.matmul(
                                        vps, ones_row[:1, :klen],
                                        bv_sb[:1, c0:c0 + 384],
                                        start=False, stop=True)
                                nc.scalar.activation(
                                    dst3[:, half * 6:(half + 1) * 6, :],
                                    vps.rearrange("p (h c) -> p h c", c=DK),
                                    AF.Copy)

            # ---------------- P2 + P3 fused per batch element --------------
            with tc.tile_pool(name="ph23", bufs=1) as P23, \
                 tc.tile_pool(name="att_sb", bufs=1) as Pat:
                wo_sb = [P23.tile([128, D], f16, name=f"wo_sb{k}")
                         for k in range(ND)]
                for k in range(ND):
                    nc.sync.dma_start(wo_sb[k],
                                      w_d["wo"][k * 128:(k + 1) * 128, :])

                for b in range(BL):
                    base = b * T
                    oT16 = [Pat.tile([128, T], f16, name=f"oT16_{b}_{hp}",
                                     tag="oT16", bufs=2 * ND + 4)
                            for hp in range(ND)]
                    # denominators: head h at partition 32*(h%4), col h//4
                    dn = Pat.tile([128, 3 * T], f32, name="dn", tag="dn",
                                  bufs=2)
                    otfs = []
                    for hp in range(ND):
                        # --- scoresT (3-term fp16, 2 heads row-packed) ---
                        eT = []
                        for kc, (koff, klen) in enumerate(KEYCHUNKS):
                            e_t = Pat.tile([128, 2 * T], f16, name="e_t",
                                           tag="eT", bufs=8)
                            fl = Pat.tile([128, 2 * T], f32, name="fl",
                                          tag="fl", bufs=6)
                            for hl in range(2):
                                pb = 64 * hl
                                ks = slice(base + koff, base + koff + klen)
                                qs = slice(base, base + T)
                                sc = ptile("sc")
                                nc.tensor.matmul(
                                    sc[:klen, :T],
                                    kT[hp][pb:pb + 64, ks],
                                    qT[hp][pb:pb + 64, qs],
                                    start=True, stop=True,
                                    tile_position=(pb, 0))
                                nc.vector.tensor_scalar(
                                    fl[:klen, hl * T:(hl + 1) * T],
                                    sc[:klen, :T],
                                    -0.5, MAGIC, OP.add, OP.add)
                            nc.scalar.activation(
                                e_t[:klen, :], fl[:klen, :], AF.Exp,
                                bias=negmagic[:klen, :1])
                            eT.append(e_t)

                        # --- attn @ v (fp16); col 64 = ones -> denominator --
                        otf = Pat.tile([128, T], f32, name="otf", tag="otf",
                                       bufs=2 * ND)
                        for hl in range(2):
                            h = 2 * hp + hl
                            op_ = ptile("oT")
                            for kc, (koff, klen) in enumerate(KEYCHUNKS):
                                nc.tensor.matmul(
                                    op_[0:HV, :T],
                                    v16e[2 * b + kc][:klen,
                                                     h * HV:(h + 1) * HV],
                                    eT[kc][:klen, hl * T:(hl + 1) * T],
                                    start=(kc == 0),
                                    stop=(kc == len(KEYCHUNKS) - 1))
                            pbase = 32 * (h % 4)
                            cb = (h // 4) * T
                            nc.vector.tensor_copy(
                                dn[pbase:pbase + 1, cb:cb + T],
                                op_[64:65, :T])
                            if hl == 0:
                                nc.scalar.activation(otf[0:64, :],
                                                     op_[0:64, :T], AF.Copy)
                            else:
                                nc.vector.tensor_copy(otf[64:128, :],
                                                      op_[0:64, :T])
                        otfs.append(otf)

                    # --- normalize: batched recip -> PE broadcast -> mul ---
                    rdf = Pat.tile([128, 3 * T], f32, name="rdf", tag="rdf",
                                   bufs=2)
                    rd16 = Pat.tile([128, 3 * T], f16, name="rd16",
                                    tag="rd16", bufs=2)
                    nc.vector.reciprocal(rdf, dn)
                    nc.vector.tensor_copy(rd16, rdf)
                    for hp in range(ND):
                        bc = ptile("bc")
                        for hl in range(2):
                            h = 2 * hp + hl
                            pbase = 32 * (h % 4)
                            cb = (h // 4) * T
                            nc.tensor.matmul(
                                bc[64 * hl:64 * hl + 64, :T],
                                ones_row[pbase:pbase + 1, :64],
                                rd16[pbase:pbase + 1, cb:cb + T],
                                start=True, stop=True,
                                tile_position=(pbase, 64 * hl))
                        nc.vector.tensor_tensor(oT16[hp], otfs[hp],
                                                bc[:, :T], OP.mult)

                    # --- P3: final projection (+bias) + store ---
                    for (roff, rn) in ((0, 128), (128, T - 128)):
                        fs = Pat.tile([128, D], f32, name="fs", tag="fs",
                                      bufs=4)
                        for half in range(2):
                            c0 = half * 384
                            fp_ = ptile("fp")
                            for d in range(ND):
                                nc.tensor.matmul(
                                    fp_[:rn, :384],
                                    oT16[d][:, roff:roff + rn],
                                    wo_sb[d][:, c0:c0 + 384],
                                    start=(d == 0),
                                    stop=(d == ND - 1 and not has_bias))
                            if has_bias:
                                nc.tensor.matmul(
                                    fp_[:rn, :384], ones_row[:1, :rn],
                                    bo_sb[:1, c0:c0 + 384],
                                    start=False, stop=True)
                            nc.vector.tensor_copy(fs[:rn, c0:c0 + 384],
                                                  fp_[:rn, :384])
                        nc.sync.dma_start(
                            out_d[base + roff:base + roff + rn, :], fs[:rn, :])

    nc.compile()
    return nc


def _split16(a):
    hi = a.astype(np.float16)
    lo = (a - hi.astype(np.float32)).astype(np.float16)
    return hi, lo


def _prep_weights(Wq, bq, Wk, bk, Wv, bv, Wo, bo, has_bias):
    f32 = np.float32
    wq = np.asarray(Wq, f32) * f32(0.125)
    wk = np.asarray(Wk, f32)
    wq_hi, wq_lo = _split16(wq)
    wk_hi, wk_lo = _split16(wk)
    w = {
        "wq_hi": wq_hi, "wq_lo": wq_lo,
        "wk_hi": wk_hi, "wk_lo": wk_lo,
        "wv": np.asarray(Wv, f32).astype(np.float16),
        "wo": np.asarray(Wo, f32).astype(np.float16),
    }
    if has_bias:
        w["bq"] = (np.asarray(bq, f32) * f32(0.125)).astype(
            np.float16).reshape(1, D)
        w["bk"] = np.asarray(bk, f32).astype(np.float16).reshape(1, D)
        w["bv"] = np.asarray(bv, f32).astype(np.float16).reshape(1, D)
        w["bo"] = np.asarray(bo, f32).astype(np.float16).reshape(1, D)
    return w


def kernel(x, Wq, bq, Wk, bk, Wv, bv, Wo, bo):
    from concourse import bass_utils

    has_bias = any(float(np.abs(np.asarray(v)).max()) != 0.0
                   for v in (bq, bk, bv, bo))
    key = ("nc", has_bias)
    if key not in _CACHE:
        _CACHE[key] = _build(has_bias)
    nc = _CACHE[key]

    x = np.asarray(x, np.float32)
    w = _prep_weights(Wq, bq, Wk, bk, Wv, bv, Wo, bo, has_bias)
    in_maps = []
    for c in range(NCORES):
        m = dict(w)
        m["x"] = np.ascontiguousarray(
            x[c * BL:(c + 1) * BL].reshape(R, D))
        in_maps.append(m)

    res = bass_utils.run_bass_kernel_spmd(nc, in_maps, list(range(NCORES)))
    out = np.concatenate(
        [res.results[c]["out"].reshape(BL, T, D) for c in range(NCORES)],
        axis=0)
    return out.astype(np.float32)
